# revision 7
# baseline (speedup 1.0000x reference)
"""Per-batch sparse attention kernel for Trainium2 (8 NeuronCores).

Math (per batch b):
  d2[i,j]  = |x_i - c_j|^2          (integer xyz coords)
  d        = max(sqrt(d2), 0.1)
  attn     = softmax_j(-d)          (over the batch's centroids only)
  out[i,j] = clip((f_i . g_j) * attn[i,j], -10, 10)
Cross-batch entries of the dense [N, 256] output are -inf and are filled on
the host; each core computes one batch's [N_b, 32] block.

One fp16 matmul per 128-cluster chunk computes BOTH the feature scores and
d2 via an augmented contraction (K = 64 + 7):
  lhsT = [feats^T ; x ; y ; z ; hi(|x|^2) ; lo(|x|^2) ; 256 ; 1]   [71, 128]
  rhs  = [cen_feats^T | (-2c ; 256 ; 1 ; hi(|c|^2) ; lo(|c|^2))]   [71, 64]
d2 is EXACT: every aug operand is an integer <= 256 (bf16-exact, 8-bit
mantissa) and all products/sums stay < 2^24 in the fp32 accumulator.  The
feature dot products are bf16-rounded (~1e-3 relative) which is well inside
tolerance.

Clusters are processed in superblocks of SB_BANKS PSUM banks (8 chunks of 128
per bank).  Within a superblock, cluster q sits at (partition p, group g) with
q = p*ngrp + g — the host permutes cluT columns accordingly — so the output
tile [128, ngrp, m] DMAs to DRAM rows in natural order with ngrp*m*4-byte
contiguous runs per partition.
"""

import sys

for _p in ("/opt/trn_rl_repo", "/opt/trn_rl_repo/pypackages"):
    if _p not in sys.path:
        sys.path.insert(0, _p)

import numpy as np

N_BATCH = 8
D_FEAT = 64
K_CONTRACT = D_FEAT + 7  # 71 bf16 contraction rows
CHUNK = 128              # clusters per matmul (PSUM partition dim)
SB_BANKS = 2             # PSUM banks per superblock
FAR_HI = 16128.0         # hi(|c|^2) for dummy centroids -> d2 ~ 4.1e6, attn 0


def _superblocks(n_pad: int, bank_clusters: int):
    sbs = []
    pos = 0
    while pos < n_pad:
        nb = min(SB_BANKS, (n_pad - pos) // bank_clusters)
        sbs.append((pos, nb))
        pos += nb * bank_clusters
    return sbs


def build_program(n_pad: int, m_cen: int):
    """Bass program for per-core shapes
    cluT [71, n_pad] bf16, rhs [71, 2*m_cen] bf16 -> out [n_pad, m_cen] f32."""
    import concourse.bacc as bacc
    import concourse.tile as tile
    from concourse import mybir

    f32 = mybir.dt.float32
    bf16 = mybir.dt.float16
    grp_bank = 512 // (2 * m_cen)       # chunks per PSUM bank (8 for m_cen=32)
    bank_clusters = CHUNK * grp_bank    # 1024
    assert n_pad % bank_clusters == 0

    nc = bacc.Bacc("TRN2", target_bir_lowering=False, debug=False,
                   num_devices=N_BATCH)
    cluT = nc.dram_tensor("cluT", [K_CONTRACT, n_pad], bf16,
                          kind="ExternalInput").ap()
    rhs = nc.dram_tensor("rhs", [K_CONTRACT, 2 * m_cen], bf16,
                         kind="ExternalInput").ap()
    out = nc.dram_tensor("out", [n_pad, m_cen], f32,
                         kind="ExternalOutput").ap()

    AF = mybir.ActivationFunctionType
    ALU = mybir.AluOpType
    AX = mybir.AxisListType

    with tile.TileContext(nc) as tc:
        with (
            tc.tile_pool(name="const", bufs=1) as constp,
            tc.tile_pool(name="inp", bufs=3) as inp,
            tc.tile_pool(name="ps", bufs=2, space="PSUM") as psp,
            tc.tile_pool(name="work", bufs=2) as wp,
            tc.tile_pool(name="outp", bufs=3) as outp,
        ):
            rhs_sb = constp.tile([K_CONTRACT, 2 * m_cen], bf16)
            nc.sync.dma_start(out=rhs_sb, in_=rhs)

            bias_zero = constp.tile([CHUNK, 1], f32)
            nc.vector.memset(bias_zero[:], 0.0)

            for start, nb in _superblocks(n_pad, bank_clusters):
                ngrp = nb * grp_bank            # 16 (or 8 for the tail)
                blk = nb * bank_clusters        # 2048 / 1024

                x = inp.tile([K_CONTRACT, blk], bf16, tag="x")
                nc.sync.dma_start(out=x[:], in_=cluT[:, start:start + blk])

                ps = psp.tile([CHUNK, ngrp, 2 * m_cen], f32, tag="ps")
                for g in range(ngrp):
                    nc.tensor.matmul(
                        ps[:, g, :],
                        x[:, g * CHUNK:(g + 1) * CHUNK],
                        rhs_sb[:],
                        start=True, stop=True,
                    )

                feat_v = ps[:, :, 0:m_cen]        # [128, ngrp, m] strided PSUM
                d2_v = ps[:, :, m_cen:2 * m_cen]  # [128, ngrp, m] strided PSUM

                # exact clamp: max(d2, 0.01), then d = sqrt(.)
                d2c = wp.tile([CHUNK, ngrp, m_cen], f32, tag="d2c")
                nc.vector.tensor_scalar_max(d2c[:], d2_v, 0.01)
                sd = wp.tile([CHUNK, ngrp, m_cen], f32, tag="sd")
                nc.scalar.activation(sd[:], d2c[:], AF.Sqrt,
                                     bias=bias_zero[:], scale=1.0)

                # row (per-cluster) min over the m centroids
                m_t = wp.tile([CHUNK, ngrp], f32, tag="m")
                nc.vector.tensor_reduce(m_t[:], sd[:], axis=AX.X, op=ALU.min)
                # e = d - m  >= 0
                e_t = wp.tile([CHUNK, ngrp, m_cen], f32, tag="e")
                nc.vector.tensor_tensor(
                    e_t[:], sd[:],
                    m_t[:].unsqueeze(2).to_broadcast((CHUNK, ngrp, m_cen)),
                    op=ALU.subtract,
                )
                # p = exp(-e)
                p_t = wp.tile([CHUNK, ngrp, m_cen], f32, tag="p")
                nc.scalar.activation(p_t[:], e_t[:], AF.Exp,
                                     bias=bias_zero[:], scale=-1.0)
                # s = sum_j p ; r = 1/s
                s_t = wp.tile([CHUNK, ngrp], f32, tag="s")
                nc.vector.tensor_reduce(s_t[:], p_t[:], axis=AX.X, op=ALU.add)
                r_t = wp.tile([CHUNK, ngrp], f32, tag="r")
                nc.vector.reciprocal(r_t[:], s_t[:])
                # t = p * featscore   (DVE: reads PSUM)
                t_t = wp.tile([CHUNK, ngrp, m_cen], f32, tag="t")
                nc.vector.tensor_mul(t_t[:], p_t[:], feat_v)
                # u = t * r ; clip    (GPSIMD: keeps DVE free)
                u_t = wp.tile([CHUNK, ngrp, m_cen], f32, tag="u")
                nc.gpsimd.tensor_mul(
                    u_t[:], t_t[:],
                    r_t[:].unsqueeze(2).to_broadcast((CHUNK, ngrp, m_cen)),
                )
                o_t = outp.tile([CHUNK, ngrp, m_cen], f32, tag="o")
                nc.gpsimd.tensor_scalar(
                    o_t[:], u_t[:], 10.0, -10.0,
                    op0=ALU.min, op1=ALU.max,
                )

                # cluster q = p*ngrp + g -> rows land in natural order with
                # ngrp*m contiguous floats per partition
                nc.sync.dma_start(
                    out=out[start:start + blk, :].rearrange(
                        "(p g) m -> p g m", p=CHUNK),
                    in_=o_t[:],
                )

    nc.compile()
    return nc


def make_core_inputs(idx_coords, idx_feats, cen_coords, cen_feats,
                     n_pad: int, m_cen: int):
    """Host-side packing of one batch's shard into bf16 cluT / rhs arrays."""
    bf = np.float16
    n_b = idx_feats.shape[0]
    m_b = cen_feats.shape[0]
    xyz = idx_coords.astype(np.float32)
    cxyz = cen_coords.astype(np.float32)
    x2 = (xyz * xyz).sum(1)
    c2 = (cxyz * cxyz).sum(1)

    cluT = np.zeros((K_CONTRACT, n_pad), bf)
    cluT[:D_FEAT, :n_b] = idx_feats.T.astype(bf)
    cluT[D_FEAT:D_FEAT + 3, :n_b] = xyz.T.astype(bf)
    cluT[D_FEAT + 3, :n_b] = np.floor(x2 / 256.0).astype(bf)   # hi(|x|^2)
    cluT[D_FEAT + 4, :n_b] = np.mod(x2, 256.0).astype(bf)      # lo(|x|^2)
    cluT[D_FEAT + 5, :n_b] = np.float32(256.0).astype(bf)
    cluT[D_FEAT + 6, :n_b] = np.float32(1.0).astype(bf)

    # permute columns per superblock: col g*128+p <- cluster p*ngrp+g
    grp_bank = 512 // (2 * m_cen)
    bank_clusters = CHUNK * grp_bank
    for start, nb in _superblocks(n_pad, bank_clusters):
        ngrp = nb * grp_bank
        blk = nb * bank_clusters
        seg = cluT[:, start:start + blk]
        cluT[:, start:start + blk] = (
            seg.reshape(K_CONTRACT, CHUNK, ngrp)
            .swapaxes(1, 2)
            .reshape(K_CONTRACT, blk)
        )

    rhs = np.zeros((K_CONTRACT, 2 * m_cen), bf)
    rhs[:D_FEAT, :m_b] = cen_feats.T.astype(bf)
    # dummy centroids sit very far away -> attn exactly 0
    rhs[D_FEAT + 5, m_cen:2 * m_cen] = np.float32(FAR_HI).astype(bf)
    rhs[D_FEAT:D_FEAT + 3, m_cen:m_cen + m_b] = (-2.0 * cxyz.T).astype(bf)
    rhs[D_FEAT + 3, m_cen:m_cen + m_b] = np.float32(256.0).astype(bf)
    rhs[D_FEAT + 4, m_cen:m_cen + m_b] = np.float32(1.0).astype(bf)
    rhs[D_FEAT + 5, m_cen:m_cen + m_b] = np.floor(c2 / 256.0).astype(bf)
    rhs[D_FEAT + 6, m_cen:m_cen + m_b] = np.mod(c2, 256.0).astype(bf)
    return {"cluT": cluT, "rhs": rhs}


def kernel(cluster_coords, cluster_feats, centroid_coords, centroid_feats):
    from concourse.bass_utils import run_bass_kernel_spmd

    cc = np.asarray(cluster_coords)
    cf = np.ascontiguousarray(np.asarray(cluster_feats), dtype=np.float32)
    ec = np.asarray(centroid_coords)
    ef = np.ascontiguousarray(np.asarray(centroid_feats), dtype=np.float32)
    n_clusters = cc.shape[0]
    n_cent = ec.shape[0]

    batch = cc[:, 0]
    cen_batch = ec[:, 0]
    idx_by_b = [np.where(batch == b)[0] for b in range(N_BATCH)]
    cols_by_b = [np.where(cen_batch == b)[0] for b in range(N_BATCH)]
    m_cen = max(len(c) for c in cols_by_b)
    assert 512 % (2 * m_cen) == 0 or m_cen <= 128, m_cen
    grp_bank = 512 // (2 * m_cen)
    bank_clusters = CHUNK * grp_bank
    n_max = max(len(i) for i in idx_by_b)
    n_pad = ((n_max + bank_clusters - 1) // bank_clusters) * bank_clusters

    nc = build_program(n_pad, m_cen)

    in_maps = [
        make_core_inputs(cc[idx_by_b[b]][:, 1:], cf[idx_by_b[b]],
                         ec[cols_by_b[b]][:, 1:], ef[cols_by_b[b]],
                         n_pad, m_cen)
        for b in range(N_BATCH)
    ]
    global _LAST
    _LAST = {"nc": nc, "in_maps": in_maps}
    res = run_bass_kernel_spmd(nc, in_maps, core_ids=list(range(N_BATCH)))

    out_full = np.full((n_clusters, n_cent), -np.inf, dtype=np.float32)
    for b in range(N_BATCH):
        shard = res.results[b]["out"]
        rows = idx_by_b[b]
        cols = cols_by_b[b]
        out_full[np.ix_(rows, cols)] = shard[:len(rows), :len(cols)]
    return out_full


# revision 8
# speedup vs baseline: 1.6465x; 1.6465x over previous
"""Per-batch sparse attention kernel for Trainium2 (8 NeuronCores).

Math (per batch b):
  d2[i,j]  = |x_i - c_j|^2          (integer xyz coords)
  d        = max(sqrt(d2), 0.1)
  attn     = softmax_j(-d)          (over the batch's centroids only)
  out[i,j] = clip((f_i . g_j) * attn[i,j], -10, 10)
Cross-batch entries of the dense [N, 256] output are -inf and are filled on
the host; each core computes one batch's [N_b, 32] block.

One fp16 matmul per 128-cluster chunk computes BOTH the feature scores and
d2 via an augmented contraction (K = 64 + 7):
  lhsT = [feats^T ; x ; y ; z ; hi(|x|^2) ; lo(|x|^2) ; 256 ; 1]   [71, 128]
  rhs  = [cen_feats^T | (-2c ; 256 ; 1 ; hi(|c|^2) ; lo(|c|^2))]   [71, 64]
d2 is EXACT: every aug operand is an integer <= 2048 (fp16-exact) and all
products/sums stay < 2^24 in the fp32 PSUM accumulator.  Only the feature
dot products are fp16-rounded (~1e-3 relative).

Layout: global permutation cluster q = p*G + g (G = n_pad/128): column
g*128+p of cluT holds cluster p*G+g, so PSUM partition p / group g maps to
DRAM row p*G+g.  The output accumulates in one persistent SBUF buffer
[128, G, 32] and drains in a few large DMAs whose per-partition runs are
contiguous multi-KB blocks; rows come out in natural order.
"""

import sys

for _p in ("/opt/trn_rl_repo", "/opt/trn_rl_repo/pypackages"):
    if _p not in sys.path:
        sys.path.insert(0, _p)

import numpy as np

N_BATCH = 8
D_FEAT = 64
K_CONTRACT = D_FEAT + 7  # 71 fp16 contraction rows
CHUNK = 128              # clusters per matmul (PSUM partition dim)
SB_BANKS = 4             # PSUM banks per superblock
FAR_HI = 16128.0         # hi(|c|^2) for dummy centroids -> d2 ~ 4.1e6, attn 0


def _superblocks(n_pad: int, bank_clusters: int):
    sbs = []
    pos = 0
    while pos < n_pad:
        nb = min(SB_BANKS, (n_pad - pos) // bank_clusters)
        sbs.append((pos, nb))
        pos += nb * bank_clusters
    return sbs


def build_program(n_pad: int, m_cen: int):
    """Bass program for per-core shapes
    cluT [71, n_pad] fp16, rhs [71, 2*m_cen] fp16 -> out [n_pad, m_cen] f32."""
    import concourse.bacc as bacc
    import concourse.tile as tile
    from concourse import mybir

    f32 = mybir.dt.float32
    f16 = mybir.dt.float16
    grp_bank = 512 // (2 * m_cen)       # chunks per PSUM bank (8 for m_cen=32)
    bank_clusters = CHUNK * grp_bank    # 1024
    assert n_pad % bank_clusters == 0
    G = n_pad // CHUNK                  # total groups (chunks)

    nc = bacc.Bacc("TRN2", target_bir_lowering=False, debug=False,
                   num_devices=N_BATCH)
    cluT = nc.dram_tensor("cluT", [K_CONTRACT, n_pad], f16,
                          kind="ExternalInput").ap()
    rhs = nc.dram_tensor("rhs", [K_CONTRACT, 2 * m_cen], f16,
                         kind="ExternalInput").ap()
    out = nc.dram_tensor("out", [n_pad, m_cen], f32,
                         kind="ExternalOutput").ap()

    AF = mybir.ActivationFunctionType
    ALU = mybir.AluOpType
    AX = mybir.AxisListType

    sbs = _superblocks(n_pad, bank_clusters)
    # drain the output buffer in ~3 large DMAs, after sb 1, 3, last
    drain_after = {1: (0, 0), len(sbs) - 1: (0, 0)}
    if len(sbs) > 3:
        drain_after = {1, 3, len(sbs) - 1}
    else:
        drain_after = {len(sbs) - 1}

    with tile.TileContext(nc) as tc:
        with (
            tc.tile_pool(name="const", bufs=1) as constp,
            tc.tile_pool(name="bigio", bufs=1) as bigio,
            tc.tile_pool(name="ps", bufs=2, space="PSUM") as psp,
            tc.tile_pool(name="work", bufs=2) as wp,
        ):
            rhs_sb = constp.tile([K_CONTRACT, 2 * m_cen], f16)
            nc.sync.dma_start(out=rhs_sb, in_=rhs)
            bias_zero = constp.tile([CHUNK, 1], f32)
            nc.vector.memset(bias_zero[:], 0.0)

            big_x = bigio.tile([K_CONTRACT, n_pad], f16)
            big_o = bigio.tile([CHUNK, G, m_cen], f32)
            out_v = out.rearrange("(p g) m -> p g m", p=CHUNK)  # [128, G, m]

            drained = 0
            for isb, (start, nb) in enumerate(sbs):
                ngrp = nb * grp_bank            # 32 (8 for the tail)
                blk = nb * bank_clusters        # 4096 / 1024
                goff = start // CHUNK

                # input chunk for this superblock (SWDGE spreads engines)
                nc.gpsimd.dma_start(out=big_x[:, start:start + blk],
                                    in_=cluT[:, start:start + blk])

                ps = psp.tile([CHUNK, SB_BANKS * grp_bank, 2 * m_cen], f32,
                              tag="ps")
                for g in range(ngrp):
                    nc.tensor.matmul(
                        ps[:, g, :],
                        big_x[:, start + g * CHUNK:start + (g + 1) * CHUNK],
                        rhs_sb[:],
                        start=True, stop=True,
                    )

                feat_v = ps[:, 0:ngrp, 0:m_cen]
                d2_v = ps[:, 0:ngrp, m_cen:2 * m_cen]

                # exact clamp: max(d2, 0.01), then d = sqrt(.)
                d2c = wp.tile([CHUNK, ngrp, m_cen], f32, tag="d2c")
                nc.vector.tensor_scalar_max(d2c[:], d2_v, 0.01)
                sd = wp.tile([CHUNK, ngrp, m_cen], f32, tag="sd")
                nc.scalar.activation(sd[:], d2c[:], AF.Sqrt,
                                     bias=bias_zero[:], scale=1.0)

                m_t = wp.tile([CHUNK, ngrp], f32, tag="m")
                nc.vector.tensor_reduce(m_t[:], sd[:], axis=AX.X, op=ALU.min)
                e_t = wp.tile([CHUNK, ngrp, m_cen], f32, tag="e")
                nc.vector.tensor_tensor(
                    e_t[:], sd[:],
                    m_t[:].unsqueeze(2).to_broadcast((CHUNK, ngrp, m_cen)),
                    op=ALU.subtract,
                )
                p_t = wp.tile([CHUNK, ngrp, m_cen], f32, tag="p")
                nc.scalar.activation(p_t[:], e_t[:], AF.Exp,
                                     bias=bias_zero[:], scale=-1.0)
                s_t = wp.tile([CHUNK, ngrp], f32, tag="s")
                nc.vector.tensor_reduce(s_t[:], p_t[:], axis=AX.X, op=ALU.add)
                r_t = wp.tile([CHUNK, ngrp], f32, tag="r")
                nc.vector.reciprocal(r_t[:], s_t[:])
                t_t = wp.tile([CHUNK, ngrp, m_cen], f32, tag="t")
                nc.vector.tensor_mul(t_t[:], p_t[:], feat_v)
                u_t = wp.tile([CHUNK, ngrp, m_cen], f32, tag="u")
                nc.gpsimd.tensor_mul(
                    u_t[:], t_t[:],
                    r_t[:].unsqueeze(2).to_broadcast((CHUNK, ngrp, m_cen)),
                )
                # clip straight into the persistent output buffer
                nc.gpsimd.tensor_scalar(
                    big_o[:, goff:goff + ngrp, :], u_t[:], 10.0, -10.0,
                    op0=ALU.min, op1=ALU.max,
                )

                if isb in drain_after:
                    gend = goff + ngrp
                    nc.sync.dma_start(
                        out=out_v[:, drained:gend, :],
                        in_=big_o[:, drained:gend, :],
                    )
                    drained = gend

    nc.compile()
    return nc


def make_core_inputs(idx_coords, idx_feats, cen_coords, cen_feats,
                     n_pad: int, m_cen: int):
    """Host-side packing of one batch's shard into fp16 cluT / rhs arrays."""
    bf = np.float16
    n_b = idx_feats.shape[0]
    m_b = cen_feats.shape[0]
    xyz = idx_coords.astype(np.float32)
    cxyz = cen_coords.astype(np.float32)
    x2 = (xyz * xyz).sum(1)
    c2 = (cxyz * cxyz).sum(1)

    cluT = np.zeros((K_CONTRACT, n_pad), bf)
    cluT[:D_FEAT, :n_b] = idx_feats.T.astype(bf)
    cluT[D_FEAT:D_FEAT + 3, :n_b] = xyz.T.astype(bf)
    cluT[D_FEAT + 3, :n_b] = np.floor(x2 / 256.0).astype(bf)   # hi(|x|^2)
    cluT[D_FEAT + 4, :n_b] = np.mod(x2, 256.0).astype(bf)      # lo(|x|^2)
    cluT[D_FEAT + 5, :n_b] = np.float32(256.0).astype(bf)
    cluT[D_FEAT + 6, :n_b] = np.float32(1.0).astype(bf)

    # global permutation: col g*128+p <- cluster p*G+g
    G = n_pad // CHUNK
    cluT = np.ascontiguousarray(
        cluT.reshape(K_CONTRACT, CHUNK, G).swapaxes(1, 2)
        .reshape(K_CONTRACT, n_pad)
    )

    rhs = np.zeros((K_CONTRACT, 2 * m_cen), bf)
    rhs[:D_FEAT, :m_b] = cen_feats.T.astype(bf)
    # dummy centroids sit very far away -> attn exactly 0
    rhs[D_FEAT + 5, m_cen:2 * m_cen] = np.float32(FAR_HI).astype(bf)
    rhs[D_FEAT:D_FEAT + 3, m_cen:m_cen + m_b] = (-2.0 * cxyz.T).astype(bf)
    rhs[D_FEAT + 3, m_cen:m_cen + m_b] = np.float32(256.0).astype(bf)
    rhs[D_FEAT + 4, m_cen:m_cen + m_b] = np.float32(1.0).astype(bf)
    rhs[D_FEAT + 5, m_cen:m_cen + m_b] = np.floor(c2 / 256.0).astype(bf)
    rhs[D_FEAT + 6, m_cen:m_cen + m_b] = np.mod(c2, 256.0).astype(bf)
    return {"cluT": cluT, "rhs": rhs}


def kernel(cluster_coords, cluster_feats, centroid_coords, centroid_feats):
    from concourse.bass_utils import run_bass_kernel_spmd

    cc = np.asarray(cluster_coords)
    cf = np.ascontiguousarray(np.asarray(cluster_feats), dtype=np.float32)
    ec = np.asarray(centroid_coords)
    ef = np.ascontiguousarray(np.asarray(centroid_feats), dtype=np.float32)
    n_clusters = cc.shape[0]
    n_cent = ec.shape[0]

    batch = cc[:, 0]
    cen_batch = ec[:, 0]
    idx_by_b = [np.where(batch == b)[0] for b in range(N_BATCH)]
    cols_by_b = [np.where(cen_batch == b)[0] for b in range(N_BATCH)]
    m_cen = max(len(c) for c in cols_by_b)
    assert 512 % (2 * m_cen) == 0 or m_cen <= 128, m_cen
    grp_bank = 512 // (2 * m_cen)
    bank_clusters = CHUNK * grp_bank
    n_max = max(len(i) for i in idx_by_b)
    n_pad = ((n_max + bank_clusters - 1) // bank_clusters) * bank_clusters

    nc = build_program(n_pad, m_cen)

    in_maps = [
        make_core_inputs(cc[idx_by_b[b]][:, 1:], cf[idx_by_b[b]],
                         ec[cols_by_b[b]][:, 1:], ef[cols_by_b[b]],
                         n_pad, m_cen)
        for b in range(N_BATCH)
    ]
    global _LAST
    _LAST = {"nc": nc, "in_maps": in_maps}
    res = run_bass_kernel_spmd(nc, in_maps, core_ids=list(range(N_BATCH)))

    out_full = np.full((n_clusters, n_cent), -np.inf, dtype=np.float32)
    for b in range(N_BATCH):
        shard = res.results[b]["out"]
        rows = idx_by_b[b]
        cols = cols_by_b[b]
        out_full[np.ix_(rows, cols)] = shard[:len(rows), :len(cols)]
    return out_full
